# revision 20
# baseline (speedup 1.0000x reference)
"""Bass/Trainium2 distributed kernel for nn_BiDirectionalLoss.

Reference computation:
    feature1 = feat[:1024], feature2 = feat[1024:]
    dis = cdist(f1, f2)                                   # [B, B]
    half-1: row-wise masked max/argmax & min/argmin of dis over columns
    half-2: identical with roles swapped -> column-wise stats of the SAME dis
    cross  = mean(relu(furthest_pos - closest_neg + 0.5))
    intra  = mean(relu(0.1 - dis_intra[pos_pos, pos_neg]))
    loss   = cross1 + cross2 + 0.5 * (intra1 + intra2)

Distribution: core k owns row-block k of dis (half-1) and column-block k
(half-2).  Everything is fused into one PSUM matrix per half:

    Q[i,j] = ||f_b[j]||^2 - 2 <a_i, f_b[j]> + BIG * (lab_a[i] == lab_b[j])

built by three accumulating matmuls (all-ones x squares, -2A^T x B, and a
bf16 one-hot x one-hot product for the label mask).  Then, per row of Q:
    max  -> matched entries sit above BIG     -> furthest positive
    min  -> unmatched entries stay below BIG  -> closest negative
    max_index on max/min values -> argmax / argmin (first occurrence)
with the row-constant ||a_i||^2 added back on the [128,1] results.
The intra terms gather two feature rows per index via indirect DMA.
A single 64B-per-core AllGather + local reduce yields the scalar loss.
"""

import sys

if "/opt/trn_rl_repo" not in sys.path:
    sys.path.insert(0, "/opt/trn_rl_repo")

import numpy as np

P = 128          # partitions / block rows per core
B = 1024         # batch size
NCORES = 8
MARGIN = 0.5
NEG_OFFSET = 100000.0
INTRA_MARGIN = 0.1
EPS = 1e-12
LAMDA = 0.5
BIG = 8192.0     # label-mask offset; > any dist^2 here, exact in bf16

_CACHE = {}


def _build():
    import concourse.bacc as bacc
    import concourse.bass as bass
    import concourse.mybir as mybir
    import concourse.tile as tile

    f32 = mybir.dt.float32
    bf16 = mybir.dt.bfloat16
    i32 = mybir.dt.int32
    u32 = mybir.dt.uint32
    AF = mybir.ActivationFunctionType
    ALU = mybir.AluOpType

    nc = bacc.Bacc("TRN2", target_bir_lowering=False, debug=False,
                   num_devices=NCORES)

    # Shared (identical on every core) inputs.
    f1T_d = nc.dram_tensor("f1T", [P, B], bf16, kind="ExternalInput")
    f2T_d = nc.dram_tensor("f2T", [P, B], bf16, kind="ExternalInput")
    feat1_d = nc.dram_tensor("feat1", [B, P], bf16, kind="ExternalInput")
    feat2_d = nc.dram_tensor("feat2", [B, P], bf16, kind="ExternalInput")
    ohB0_d = nc.dram_tensor("ohB0", [P, B], bf16, kind="ExternalInput")
    ohB1_d = nc.dram_tensor("ohB1", [P, B], bf16, kind="ExternalInput")
    # Per-core (sharded) inputs.
    a0m2_d = nc.dram_tensor("a0m2", [P, P], bf16, kind="ExternalInput")
    a1m2_d = nc.dram_tensor("a1m2", [P, P], bf16, kind="ExternalInput")
    ohA0_d = nc.dram_tensor("ohA0", [P, P], bf16, kind="ExternalInput")
    ohA1_d = nc.dram_tensor("ohA1", [P, P], bf16, kind="ExternalInput")
    fblk1_d = nc.dram_tensor("fblk1", [P, P], bf16, kind="ExternalInput")
    fblk2_d = nc.dram_tensor("fblk2", [P, P], bf16, kind="ExternalInput")
    out_d = nc.dram_tensor("out", [1, 4], f32, kind="ExternalOutput")

    with tile.TileContext(nc) as tc:
        with (
            tc.tile_pool(name="sb", bufs=1) as sb,
            tc.tile_pool(name="ps", space="PSUM", bufs=1) as ps,
            tc.tile_pool(name="dr", space="DRAM", bufs=1) as dr,
        ):
            # ---- loads ----
            f1T = sb.tile([P, B], bf16, name="f1T_sb")
            f2T = sb.tile([P, B], bf16, name="f2T_sb")
            a0m2 = sb.tile([P, P], bf16, name="a0m2_sb")
            a1m2 = sb.tile([P, P], bf16, name="a1m2_sb")
            fblk1 = sb.tile([P, P], bf16, name="fblk1_sb")
            fblk2 = sb.tile([P, P], bf16, name="fblk2_sb")
            ohA = [sb.tile([P, P], bf16, name="ohA0_sb"),
                   sb.tile([P, P], bf16, name="ohA1_sb")]
            ohB = [sb.tile([P, B], bf16, name="ohB0_sb"),
                   sb.tile([P, B], bf16, name="ohB1_sb")]
            # Loads split into 512-column halves and spread over the three
            # DMA-capable queues, ordered by when the Q-matmul chain needs
            # them (half-0 chunk-0 operands first).
            c0 = slice(0, 512)
            c1 = slice(512, B)
            nc.sync.dma_start(f2T[:, c0], f2T_d[:, c0])
            nc.scalar.dma_start(a0m2[:], a0m2_d[:])
            nc.gpsimd.dma_start(ohA[0][:], ohA0_d[:])
            nc.sync.dma_start(ohB[0][:, c0], ohB0_d[:, c0])
            nc.scalar.dma_start(f2T[:, c1], f2T_d[:, c1])
            nc.gpsimd.dma_start(ohB[0][:, c1], ohB0_d[:, c1])
            nc.sync.dma_start(f1T[:, c0], f1T_d[:, c0])
            nc.scalar.dma_start(a1m2[:], a1m2_d[:])
            nc.gpsimd.dma_start(ohA[1][:], ohA1_d[:])
            nc.scalar.dma_start(f1T[:, c1], f1T_d[:, c1])
            nc.sync.dma_start(ohB[1][:, c0], ohB1_d[:, c0])
            nc.gpsimd.dma_start(ohB[1][:, c1], ohB1_d[:, c1])
            nc.scalar.dma_start(fblk1[:], fblk1_d[:])
            nc.gpsimd.dma_start(fblk2[:], fblk2_d[:])

            ones = sb.tile([P, P], bf16, name="ones_sb")
            nc.vector.memset(ones[:], 1.0)
            eps_t = sb.tile([P, 1], f32, name="eps_sb")
            nc.vector.memset(eps_t[:], EPS)
            margin_t = sb.tile([P, 1], f32, name="margin_sb")
            nc.vector.memset(margin_t[:], MARGIN)
            intram_t = sb.tile([P, 1], f32, name="intram_sb")
            nc.vector.memset(intram_t[:], INTRA_MARGIN)


            # pre-warm the Sqrt/Square activation tables (1.3us loads) so
            # they don't interrupt the pipeline later
            warm_sq = sb.tile([P, 1], f32, name="warm_sq_sb")
            nc.scalar.activation(warm_sq[:], eps_t[:], AF.Sqrt, bias=eps_t[:])
            nc.scalar.square(warm_sq[:], eps_t[:])

            # squared features (rhs of the all-ones norm matmul)
            sq1 = sb.tile([P, B], bf16, name="sq1_sb")
            sq2 = sb.tile([P, B], bf16, name="sq2_sb")
            for cs in (c0, c1):
                nc.scalar.square(sq2[:, cs], f2T[:, cs])
            for cs in (c0, c1):
                nc.scalar.square(sq1[:, cs], f1T[:, cs])

            # per-row block norms ||a_i||^2 -> [128, 1]
            na2 = []
            for s_i, fb in ((0, fblk1), (1, fblk2)):
                sqb = sb.tile([P, P], f32, name=f"sqb{s_i}_sb")
                nc.scalar.square(sqb[:], fb[:])
                nn = sb.tile([P, 1], f32, name=f"na2_{s_i}_sb")
                nc.vector.reduce_sum(nn[:], sqb[:], axis=mybir.AxisListType.X)
                na2.append(nn)

            # ---- per-half pipeline ----
            stats = sb.tile([P, 4], f32, name="stats_sb")  # [ct0, ct1, it0, it1]
            for h in range(2):
                if h == 0:   # rows of f1 vs all f2
                    am2, sqb_full, na_col = a0m2, sq2, na2[0]
                    fbT, featB_d = f2T, feat2_d
                else:        # rows of f2 vs all f1
                    am2, sqb_full, na_col = a1m2, sq1, na2[1]
                    fbT, featB_d = f1T, feat1_d

                # Q = nb2[j] - 2<a_i, b_j> + BIG*s  (accumulated in PSUM).
                # The norm-row term goes last: it needs the squares, which
                # the scalar engine computes while the first matmuls run.
                Q = ps.tile([P, B], f32, name=f"Q{h}", tag="Q", bufs=2)
                for c in range(2):
                    cs = slice(c * 512, (c + 1) * 512)
                    nc.tensor.matmul(Q[:, cs], lhsT=am2[:], rhs=fbT[:, cs],
                                     start=True, stop=False)
                    nc.tensor.matmul(Q[:, cs], lhsT=ohA[h][:],
                                     rhs=ohB[h][:, cs],
                                     start=False, stop=False)
                    nc.tensor.matmul(Q[:, cs], lhsT=ones[:],
                                     rhs=sqb_full[:, cs],
                                     start=False, stop=True)

                # row stats straight off PSUM; find-index follows its reduce
                # so the indirect gathers can start as early as possible
                maxv = sb.tile([P, 1], f32, name=f"maxv{h}_sb")
                nc.vector.tensor_reduce(maxv[:], Q[:], op=ALU.max,
                                        axis=mybir.AxisListType.X)
                idxp = sb.tile([P, 8], u32, name=f"idxp{h}_sb")
                nc.vector.max_index(idxp[:], maxv[:].to_broadcast([P, 8]), Q[:])
                gP = sb.tile([P, P], bf16, name=f"gP{h}_sb")
                nc.gpsimd.indirect_dma_start(
                    out=gP[:], out_offset=None, in_=featB_d[:],
                    in_offset=bass.IndirectOffsetOnAxis(ap=idxp[:, 0:1], axis=0))
                minv = sb.tile([P, 1], f32, name=f"minv{h}_sb")
                nc.vector.tensor_reduce(minv[:], Q[:], op=ALU.min,
                                        axis=mybir.AxisListType.X)
                idxn = sb.tile([P, 8], u32, name=f"idxn{h}_sb")
                nc.vector.max_index(idxn[:], minv[:].to_broadcast([P, 8]), Q[:])
                gN = sb.tile([P, P], bf16, name=f"gN{h}_sb")
                nc.gpsimd.indirect_dma_start(
                    out=gN[:], out_offset=None, in_=featB_d[:],
                    in_offset=bass.IndirectOffsetOnAxis(ap=idxn[:, 0:1], axis=0))

                # fp = sqrt(relu(maxv + na2 - BIG) + eps)
                # cn = sqrt(relu(minv + na2) + eps)
                nb_bias = sb.tile([P, 1], f32, name=f"nb_bias{h}_sb")
                nc.vector.tensor_scalar(nb_bias[:], na_col[:], -BIG, None,
                                        op0=ALU.add)
                fp2 = sb.tile([P, 1], f32, name=f"fp2_{h}_sb")
                nc.scalar.activation(fp2[:], maxv[:], AF.Relu, bias=nb_bias[:])
                fp = sb.tile([P, 1], f32, name=f"fp{h}_sb")
                nc.scalar.activation(fp[:], fp2[:], AF.Sqrt, bias=eps_t[:])
                cn2 = sb.tile([P, 1], f32, name=f"cn2_{h}_sb")
                nc.scalar.activation(cn2[:], minv[:], AF.Relu, bias=na_col[:])
                cn = sb.tile([P, 1], f32, name=f"cn{h}_sb")
                nc.scalar.activation(cn[:], cn2[:], AF.Sqrt, bias=eps_t[:])

                # cross term: relu(fp - cn + margin)
                cd = sb.tile([P, 1], f32, name=f"cd{h}_sb")
                nc.vector.tensor_tensor(out=cd[:], in0=fp[:], in1=cn[:],
                                        op=ALU.subtract)
                nc.scalar.activation(stats[:, h:h + 1], cd[:], AF.Relu,
                                     bias=margin_t[:])

                # intra term: distance between gathered rows, hinge.
                # diff and sum-of-squares both stay on the vector engine.
                diff = sb.tile([P, P], f32, name=f"diff{h}_sb")
                nc.vector.tensor_tensor(out=diff[:], in0=gP[:], in1=gN[:],
                                        op=ALU.subtract)
                dsq = sb.tile([P, P], f32, name=f"dsq{h}_sb")
                ssq = sb.tile([P, 1], f32, name=f"ssq{h}_sb")
                nc.scalar.activation(dsq[:], diff[:], AF.Square,
                                     accum_out=ssq[:])
                gd = sb.tile([P, 1], f32, name=f"gd{h}_sb")
                nc.scalar.activation(gd[:], ssq[:], AF.Sqrt, bias=eps_t[:])
                nc.scalar.activation(stats[:, 2 + h:3 + h], gd[:], AF.Relu,
                                     scale=-1.0, bias=intram_t[:])

            # ---- weighted partial sums + AllGather + final scalar ----
            w_ct = sb.tile([P, 1], f32, name="w_ct_sb")
            nc.vector.memset(w_ct[:], 1.0 / B)
            w_it = sb.tile([P, 1], f32, name="w_it_sb")
            nc.vector.memset(w_it[:], LAMDA / B)
            pfin = ps.tile([1, 4], f32, name="pfin")
            nc.tensor.matmul(pfin[0:1, 0:2], lhsT=w_ct[:], rhs=stats[:, 0:2],
                             start=True, stop=True)
            nc.tensor.matmul(pfin[0:1, 2:4], lhsT=w_it[:], rhs=stats[:, 2:4],
                             start=True, stop=True, skip_group_check=True)

            # Each core outputs its 4 weighted partial sums; the host-side
            # unshard adds the 8x4 partials into the scalar loss.
            part = sb.tile([1, 4], f32, name="part_sb")
            nc.scalar.copy(part[:], pfin[:])
            nc.sync.dma_start(out_d[:], part[:])

    nc.compile()
    return nc


def _get_nc():
    if "nc" not in _CACHE:
        _CACHE["nc"] = _build()
    return _CACHE["nc"]


def _in_maps(feat, label1, label2):
    import ml_dtypes
    bf = ml_dtypes.bfloat16
    feat = np.asarray(feat, dtype=np.float32)
    f1 = np.ascontiguousarray(feat[:B])
    f2 = np.ascontiguousarray(feat[B:])
    f1T = np.ascontiguousarray(f1.T)
    f2T = np.ascontiguousarray(f2.T)
    f1Tb = f1T.astype(bf)
    f2Tb = f2T.astype(bf)
    l1 = np.asarray(label1).astype(np.int64)
    l2 = np.asarray(label2).astype(np.int64)
    classes = np.arange(P)
    # one-hot encodings [class, index]; ohB side carries the BIG scale
    oh1 = (l1[None, :] == classes[:, None]).astype(bf)          # [128, B]
    oh2 = (l2[None, :] == classes[:, None]).astype(bf)
    ohB0 = np.ascontiguousarray(oh2 * bf(BIG))                  # half-0 mask
    ohB1 = np.ascontiguousarray(oh1 * bf(BIG))
    maps = []
    for k in range(NCORES):
        blk = slice(k * P, (k + 1) * P)
        maps.append({
            "f1T": f1Tb,
            "f2T": f2Tb,
            "feat1": f1.astype(bf),
            "feat2": f2.astype(bf),
            "ohB0": ohB0,
            "ohB1": ohB1,
            "a0m2": np.ascontiguousarray(-2.0 * f1T[:, blk]).astype(bf),
            "a1m2": np.ascontiguousarray(-2.0 * f2T[:, blk]).astype(bf),
            "ohA0": np.ascontiguousarray(oh1[:, blk]),
            "ohA1": np.ascontiguousarray(oh2[:, blk]),
            "fblk1": np.ascontiguousarray(f1[blk]).astype(bf),
            "fblk2": np.ascontiguousarray(f2[blk]).astype(bf),
        })
    return maps


def _run(feat, label1, label2, trace=False):
    from concourse.bass_utils import run_bass_kernel_spmd

    nc = _get_nc()
    res = run_bass_kernel_spmd(nc, _in_maps(feat, label1, label2),
                               core_ids=list(range(NCORES)), trace=trace)
    total = np.float32(0.0)
    for r in res.results:
        total += np.float32(r["out"].astype(np.float32).sum())
    return total, res.exec_time_ns


def kernel(feat, label1, label2):
    val, _ = _run(feat, label1, label2)
    return np.array(val, dtype=np.float32)


# revision 21
# speedup vs baseline: 1.0816x; 1.0816x over previous
"""Bass/Trainium2 distributed kernel for nn_BiDirectionalLoss.

Reference computation:
    feature1 = feat[:1024], feature2 = feat[1024:]
    dis = cdist(f1, f2)                                   # [B, B]
    half-1: row-wise masked max/argmax & min/argmin of dis over columns
    half-2: identical with roles swapped -> column-wise stats of the SAME dis
    cross  = mean(relu(furthest_pos - closest_neg + 0.5))
    intra  = mean(relu(0.1 - dis_intra[pos_pos, pos_neg]))
    loss   = cross1 + cross2 + 0.5 * (intra1 + intra2)

Distribution: core k owns row-block k of dis (half-1) and column-block k
(half-2).  Everything is fused into one PSUM matrix per half:

    Q[i,j] = ||f_b[j]||^2 - 2 <a_i, f_b[j]> + BIG * (lab_a[i] == lab_b[j])

built by three accumulating matmuls (all-ones x squares, -2A^T x B, and a
bf16 one-hot x one-hot product for the label mask).  Then, per row of Q:
    max  -> matched entries sit above BIG     -> furthest positive
    min  -> unmatched entries stay below BIG  -> closest negative
    max_index on max/min values -> argmax / argmin (first occurrence)
with the row-constant ||a_i||^2 added back on the [128,1] results.
The intra terms gather two feature rows per index via indirect DMA.
A single 64B-per-core AllGather + local reduce yields the scalar loss.
"""

import sys

if "/opt/trn_rl_repo" not in sys.path:
    sys.path.insert(0, "/opt/trn_rl_repo")

import numpy as np

P = 128          # partitions / block rows per core
B = 1024         # batch size
NCORES = 8
MARGIN = 0.5
NEG_OFFSET = 100000.0
INTRA_MARGIN = 0.1
EPS = 1e-12
LAMDA = 0.5
BIG = 8192.0     # label-mask offset; > any dist^2 here, exact in bf16

_CACHE = {}


def _build():
    import concourse.bacc as bacc
    import concourse.bass as bass
    import concourse.mybir as mybir
    import concourse.tile as tile

    f32 = mybir.dt.float32
    bf16 = mybir.dt.bfloat16
    i32 = mybir.dt.int32
    u32 = mybir.dt.uint32
    AF = mybir.ActivationFunctionType
    ALU = mybir.AluOpType

    nc = bacc.Bacc("TRN2", target_bir_lowering=False, debug=False,
                   num_devices=NCORES)

    # Shared (identical on every core) inputs.
    f1T_d = nc.dram_tensor("f1T", [P, B], bf16, kind="ExternalInput")
    f2T_d = nc.dram_tensor("f2T", [P, B], bf16, kind="ExternalInput")
    feat1_d = nc.dram_tensor("feat1", [B, P], bf16, kind="ExternalInput")
    feat2_d = nc.dram_tensor("feat2", [B, P], bf16, kind="ExternalInput")
    ohB0_d = nc.dram_tensor("ohB0", [P, B], bf16, kind="ExternalInput")
    ohB1_d = nc.dram_tensor("ohB1", [P, B], bf16, kind="ExternalInput")
    # Per-core (sharded) inputs.
    a0m2_d = nc.dram_tensor("a0m2", [P, P], bf16, kind="ExternalInput")
    a1m2_d = nc.dram_tensor("a1m2", [P, P], bf16, kind="ExternalInput")
    ohA0_d = nc.dram_tensor("ohA0", [P, P], bf16, kind="ExternalInput")
    ohA1_d = nc.dram_tensor("ohA1", [P, P], bf16, kind="ExternalInput")
    fblk1_d = nc.dram_tensor("fblk1", [P, P], bf16, kind="ExternalInput")
    fblk2_d = nc.dram_tensor("fblk2", [P, P], bf16, kind="ExternalInput")
    out_d = nc.dram_tensor("out", [1, 4], f32, kind="ExternalOutput")

    with tile.TileContext(nc) as tc:
        with (
            tc.tile_pool(name="sb", bufs=1) as sb,
            tc.tile_pool(name="ps", space="PSUM", bufs=1) as ps,
            tc.tile_pool(name="dr", space="DRAM", bufs=1) as dr,
        ):
            # ---- loads ----
            f1T = sb.tile([P, B], bf16, name="f1T_sb")
            f2T = sb.tile([P, B], bf16, name="f2T_sb")
            a0m2 = sb.tile([P, P], bf16, name="a0m2_sb")
            a1m2 = sb.tile([P, P], bf16, name="a1m2_sb")
            fblk1 = sb.tile([P, P], bf16, name="fblk1_sb")
            fblk2 = sb.tile([P, P], bf16, name="fblk2_sb")
            ohA = [sb.tile([P, P], bf16, name="ohA0_sb"),
                   sb.tile([P, P], bf16, name="ohA1_sb")]
            ohB = [sb.tile([P, B], bf16, name="ohB0_sb"),
                   sb.tile([P, B], bf16, name="ohB1_sb")]
            # Loads split into 512-column halves and spread over the three
            # DMA-capable queues, ordered by when the Q-matmul chain needs
            # them (half-0 chunk-0 operands first).
            c0 = slice(0, 512)
            c1 = slice(512, B)
            nc.sync.dma_start(f2T[:, c0], f2T_d[:, c0])
            nc.gpsimd.dma_start(a0m2[:], a0m2_d[:])
            nc.gpsimd.dma_start(ohA[0][:], ohA0_d[:])
            nc.sync.dma_start(ohB[0][:, c0], ohB0_d[:, c0])
            nc.gpsimd.dma_start(f2T[:, c1], f2T_d[:, c1])
            nc.sync.dma_start(ohB[0][:, c1], ohB0_d[:, c1])
            nc.sync.dma_start(f1T[:, c0], f1T_d[:, c0])
            nc.gpsimd.dma_start(a1m2[:], a1m2_d[:])
            nc.gpsimd.dma_start(ohA[1][:], ohA1_d[:])
            nc.gpsimd.dma_start(f1T[:, c1], f1T_d[:, c1])
            nc.sync.dma_start(ohB[1][:, c0], ohB1_d[:, c0])
            nc.gpsimd.dma_start(ohB[1][:, c1], ohB1_d[:, c1])
            nc.sync.dma_start(fblk1[:], fblk1_d[:])
            nc.sync.dma_start(fblk2[:], fblk2_d[:])

            ones = sb.tile([P, P], bf16, name="ones_sb")
            nc.vector.memset(ones[:], 1.0)
            eps_t = sb.tile([P, 1], f32, name="eps_sb")
            nc.vector.memset(eps_t[:], EPS)
            margin_t = sb.tile([P, 1], f32, name="margin_sb")
            nc.vector.memset(margin_t[:], MARGIN)
            intram_t = sb.tile([P, 1], f32, name="intram_sb")
            nc.vector.memset(intram_t[:], INTRA_MARGIN)


            # pre-warm the Sqrt/Square activation tables (1.3us loads) so
            # they don't interrupt the pipeline later
            warm_sq = sb.tile([P, 1], f32, name="warm_sq_sb")
            nc.scalar.activation(warm_sq[:], eps_t[:], AF.Sqrt, bias=eps_t[:])
            nc.scalar.square(warm_sq[:], eps_t[:])

            # squared features (rhs of the all-ones norm matmul)
            sq1 = sb.tile([P, B], bf16, name="sq1_sb")
            sq2 = sb.tile([P, B], bf16, name="sq2_sb")
            for cs in (c0, c1):
                nc.scalar.square(sq2[:, cs], f2T[:, cs])
            for cs in (c0, c1):
                nc.scalar.square(sq1[:, cs], f1T[:, cs])

            # per-row block norms ||a_i||^2 -> [128, 1]
            na2 = []
            for s_i, fb in ((0, fblk1), (1, fblk2)):
                sqb = sb.tile([P, P], f32, name=f"sqb{s_i}_sb")
                nc.scalar.square(sqb[:], fb[:])
                nn = sb.tile([P, 1], f32, name=f"na2_{s_i}_sb")
                nc.vector.reduce_sum(nn[:], sqb[:], axis=mybir.AxisListType.X)
                na2.append(nn)

            # ---- per-half pipeline ----
            stats = sb.tile([P, 4], f32, name="stats_sb")  # [ct0, ct1, it0, it1]
            for h in range(2):
                if h == 0:   # rows of f1 vs all f2
                    am2, sqb_full, na_col = a0m2, sq2, na2[0]
                    fbT, featB_d = f2T, feat2_d
                else:        # rows of f2 vs all f1
                    am2, sqb_full, na_col = a1m2, sq1, na2[1]
                    fbT, featB_d = f1T, feat1_d

                # Q = nb2[j] - 2<a_i, b_j> + BIG*s  (accumulated in PSUM).
                # The norm-row term goes last: it needs the squares, which
                # the scalar engine computes while the first matmuls run.
                Q = ps.tile([P, B], f32, name=f"Q{h}", tag="Q", bufs=2)
                for c in range(2):
                    cs = slice(c * 512, (c + 1) * 512)
                    nc.tensor.matmul(Q[:, cs], lhsT=am2[:], rhs=fbT[:, cs],
                                     start=True, stop=False)
                    nc.tensor.matmul(Q[:, cs], lhsT=ohA[h][:],
                                     rhs=ohB[h][:, cs],
                                     start=False, stop=False)
                    nc.tensor.matmul(Q[:, cs], lhsT=ones[:],
                                     rhs=sqb_full[:, cs],
                                     start=False, stop=True)

                # Row stats straight off PSUM.  One FIND pass serves both
                # argmax and argmin: max_index returns the first occurrence
                # of each of the 8 target values, so targets
                # [maxv, minv, minv, ...] (non-increasing) yield argmax in
                # column 0 and argmin in column 1.
                mm8 = sb.tile([P, 8], f32, name=f"mm8_{h}_sb")
                maxv = mm8[:, 0:1]
                minv = mm8[:, 1:2]
                nc.vector.tensor_reduce(maxv, Q[:], op=ALU.max,
                                        axis=mybir.AxisListType.X)
                nc.vector.tensor_reduce(minv, Q[:], op=ALU.min,
                                        axis=mybir.AxisListType.X)
                nc.vector.tensor_copy(mm8[:, 2:8],
                                      mm8[:, 1:2].to_broadcast([P, 6]))
                idx8 = sb.tile([P, 8], u32, name=f"idx8_{h}_sb")
                nc.vector.max_index(idx8[:], mm8[:], Q[:])
                idxp = idx8[:, 0:1]
                idxn = idx8[:, 1:2]
                gP = sb.tile([P, P], bf16, name=f"gP{h}_sb")
                nc.gpsimd.indirect_dma_start(
                    out=gP[:], out_offset=None, in_=featB_d[:],
                    in_offset=bass.IndirectOffsetOnAxis(ap=idxp, axis=0))
                gN = sb.tile([P, P], bf16, name=f"gN{h}_sb")
                nc.gpsimd.indirect_dma_start(
                    out=gN[:], out_offset=None, in_=featB_d[:],
                    in_offset=bass.IndirectOffsetOnAxis(ap=idxn, axis=0))

                # fp = sqrt(relu(maxv + na2 - BIG) + eps)
                # cn = sqrt(relu(minv + na2) + eps)
                nb_bias = sb.tile([P, 1], f32, name=f"nb_bias{h}_sb")
                nc.vector.tensor_scalar(nb_bias[:], na_col[:], -BIG, None,
                                        op0=ALU.add)
                fp2 = sb.tile([P, 1], f32, name=f"fp2_{h}_sb")
                nc.scalar.activation(fp2[:], maxv, AF.Relu, bias=nb_bias[:])
                fp = sb.tile([P, 1], f32, name=f"fp{h}_sb")
                nc.scalar.activation(fp[:], fp2[:], AF.Sqrt, bias=eps_t[:])
                cn2 = sb.tile([P, 1], f32, name=f"cn2_{h}_sb")
                nc.scalar.activation(cn2[:], minv, AF.Relu, bias=na_col[:])
                cn = sb.tile([P, 1], f32, name=f"cn{h}_sb")
                nc.scalar.activation(cn[:], cn2[:], AF.Sqrt, bias=eps_t[:])

                # cross term: relu(fp - cn + margin)
                cd = sb.tile([P, 1], f32, name=f"cd{h}_sb")
                nc.vector.tensor_tensor(out=cd[:], in0=fp[:], in1=cn[:],
                                        op=ALU.subtract)
                nc.scalar.activation(stats[:, h:h + 1], cd[:], AF.Relu,
                                     bias=margin_t[:])

                # intra term: distance between gathered rows, hinge.
                # diff and sum-of-squares both stay on the vector engine.
                diff = sb.tile([P, P], f32, name=f"diff{h}_sb")
                nc.vector.tensor_tensor(out=diff[:], in0=gP[:], in1=gN[:],
                                        op=ALU.subtract)
                dsq = sb.tile([P, P], f32, name=f"dsq{h}_sb")
                ssq = sb.tile([P, 1], f32, name=f"ssq{h}_sb")
                nc.scalar.activation(dsq[:], diff[:], AF.Square,
                                     accum_out=ssq[:])
                gd = sb.tile([P, 1], f32, name=f"gd{h}_sb")
                nc.scalar.activation(gd[:], ssq[:], AF.Sqrt, bias=eps_t[:])
                nc.scalar.activation(stats[:, 2 + h:3 + h], gd[:], AF.Relu,
                                     scale=-1.0, bias=intram_t[:])

            # ---- weighted partial sums + AllGather + final scalar ----
            w_ct = sb.tile([P, 1], f32, name="w_ct_sb")
            nc.vector.memset(w_ct[:], 1.0 / B)
            w_it = sb.tile([P, 1], f32, name="w_it_sb")
            nc.vector.memset(w_it[:], LAMDA / B)
            pfin = ps.tile([1, 4], f32, name="pfin")
            nc.tensor.matmul(pfin[0:1, 0:2], lhsT=w_ct[:], rhs=stats[:, 0:2],
                             start=True, stop=True)
            nc.tensor.matmul(pfin[0:1, 2:4], lhsT=w_it[:], rhs=stats[:, 2:4],
                             start=True, stop=True, skip_group_check=True)

            # Each core outputs its 4 weighted partial sums; the host-side
            # unshard adds the 8x4 partials into the scalar loss.
            part = sb.tile([1, 4], f32, name="part_sb")
            nc.scalar.copy(part[:], pfin[:])
            nc.sync.dma_start(out_d[:], part[:])

    nc.compile()
    return nc


def _get_nc():
    if "nc" not in _CACHE:
        _CACHE["nc"] = _build()
    return _CACHE["nc"]


def _in_maps(feat, label1, label2):
    import ml_dtypes
    bf = ml_dtypes.bfloat16
    feat = np.asarray(feat, dtype=np.float32)
    f1 = np.ascontiguousarray(feat[:B])
    f2 = np.ascontiguousarray(feat[B:])
    f1T = np.ascontiguousarray(f1.T)
    f2T = np.ascontiguousarray(f2.T)
    f1Tb = f1T.astype(bf)
    f2Tb = f2T.astype(bf)
    l1 = np.asarray(label1).astype(np.int64)
    l2 = np.asarray(label2).astype(np.int64)
    classes = np.arange(P)
    # one-hot encodings [class, index]; ohB side carries the BIG scale
    oh1 = (l1[None, :] == classes[:, None]).astype(bf)          # [128, B]
    oh2 = (l2[None, :] == classes[:, None]).astype(bf)
    ohB0 = np.ascontiguousarray(oh2 * bf(BIG))                  # half-0 mask
    ohB1 = np.ascontiguousarray(oh1 * bf(BIG))
    maps = []
    for k in range(NCORES):
        blk = slice(k * P, (k + 1) * P)
        maps.append({
            "f1T": f1Tb,
            "f2T": f2Tb,
            "feat1": f1.astype(bf),
            "feat2": f2.astype(bf),
            "ohB0": ohB0,
            "ohB1": ohB1,
            "a0m2": np.ascontiguousarray(-2.0 * f1T[:, blk]).astype(bf),
            "a1m2": np.ascontiguousarray(-2.0 * f2T[:, blk]).astype(bf),
            "ohA0": np.ascontiguousarray(oh1[:, blk]),
            "ohA1": np.ascontiguousarray(oh2[:, blk]),
            "fblk1": np.ascontiguousarray(f1[blk]).astype(bf),
            "fblk2": np.ascontiguousarray(f2[blk]).astype(bf),
        })
    return maps


def _run(feat, label1, label2, trace=False):
    from concourse.bass_utils import run_bass_kernel_spmd

    nc = _get_nc()
    res = run_bass_kernel_spmd(nc, _in_maps(feat, label1, label2),
                               core_ids=list(range(NCORES)), trace=trace)
    total = np.float32(0.0)
    for r in res.results:
        total += np.float32(r["out"].astype(np.float32).sum())
    return total, res.exec_time_ns


def kernel(feat, label1, label2):
    val, _ = _run(feat, label1, label2)
    return np.array(val, dtype=np.float32)
